# revision 18
# baseline (speedup 1.0000x reference)
"""3-layer GCN + global mean pool + linear head on 8 TRN2 NeuronCores.

Nodes are partitioned across 8 cores. Layers 1-2: h = g_prev @ W on the PE,
g = dinv*h into a local table, AllGather the full table, then dma_gather
instructions ROTATED ACROSS ALL 4 SWDGE QUEUES (each queue's descriptors
are generated by a different GPSIMD Q7 core pair; queues 1-3 dispatch in
~80ns and generate asynchronously, queue 0 blocks the Pool sequencer for
its generation, which paces the rotation) fetch every in-edge's source
row; DVE segment-reduces; self-loop contributions are added via an aligned
copy of the local g block (no gather rows spent on them). Gather/reduce
slots are 6 deep so round k+1's generation overlaps round k's DMA
transfers.

Layer 3 + mean-pool + linear head collapse into a single static matrix:
pooled @ lin_W = (M^T h2) (W3 lin_W) + const, where M[s,g] = sum over edges
s->d of norm(s,d)/|graph(d)| is precomputed on the host in bf16 and streamed
through the PE during layer-2's tail (98 matmuls), then one AllReduce of the
[64, 1000] partial and a tiny head matmul finish the job.
"""

import sys

sys.path.insert(0, "/opt/trn_rl_repo")

import numpy as np

C = 8
F = 64
P = 128
MAXCOL = 52
RUNCOL = 52
NSLOT = 6
QMAP = [1, 2, 3, 0]   # SWDGE queue rotation (4 Q7 pairs)
G = 1000
GP = 1024


# ---------------------------------------------------------------------------
# host schedule
# ---------------------------------------------------------------------------

def _schedule(x, edge_index, batch, n_graphs):
    N = x.shape[0]
    NPC = -(-N // C)
    NB = -(-(NPC + 1) // P)
    NPCP = NB * P
    assert 2 * NPCP <= 32768

    src = np.asarray(edge_index[0], dtype=np.int64)
    dst = np.asarray(edge_index[1], dtype=np.int64)
    batch = np.asarray(batch, dtype=np.int64)

    # degrees INCLUDE self-loops (reference semantics)
    deg = np.bincount(dst, minlength=N).astype(np.float32) + 1.0
    dinv = (1.0 / np.sqrt(deg)).astype(np.float32)
    cnt = np.bincount(batch, minlength=n_graphs).astype(np.float32)
    inv_cnt_g = (1.0 / np.maximum(cnt, 1.0)).astype(np.float32)

    owner = np.minimum(dst // NPC, C - 1)
    canon = np.zeros(N, dtype=np.int64)

    cores = []
    for c in range(C):
        lo, hi = c * NPC, min((c + 1) * NPC, N)
        nreal = hi - lo
        m = owner == c
        es, ed = src[m], dst[m] - lo
        eg = np.minimum(es // NPC, C - 1) // 2
        degq = np.zeros((4, NPCP), dtype=np.int64)
        for q in range(4):
            dq = ed[eg == q]
            if dq.size:
                degq[q] += np.bincount(dq, minlength=NPCP)[:NPCP]
        orders = [np.argsort(-degq[q], kind="stable") for q in range(4)]
        ranks = []
        for q in range(4):
            r = np.empty(NPCP, dtype=np.int64)
            r[orders[q]] = np.arange(NPCP)
            ranks.append(r)
        canon[lo:hi] = ranks[3][:nreal]
        cores.append(dict(lo=lo, hi=hi, nreal=nreal, es=es, ed=ed, eg=eg,
                          degq=degq, orders=orders, ranks=ranks))

    # unified per-(q, block) gather widths
    Dqb = np.zeros((4, NB), dtype=np.int64)
    for c in range(C):
        cc = cores[c]
        for q in range(4):
            srt = -np.sort(-cc["degq"][q])
            for b in range(NB):
                Dqb[q, b] = max(Dqb[q, b], 1, int(srt[b * P]))

    # shared instruction layout
    instrs = []

    def chop(kind, qq, blocks_D):
        redops, coff = [], 0
        for b, D in blocks_D:
            for r0 in range(0, int(D), RUNCOL):
                rl = min(RUNCOL, int(D) - r0)
                if coff + rl > MAXCOL:
                    instrs.append(dict(kind=kind, q=qq, ncol=coff, red=redops))
                    redops, coff = [], 0
                redops.append(dict(b=b, off=coff, D=rl, acc=r0 > 0, r0=r0))
                coff += rl
        if coff:
            instrs.append(dict(kind=kind, q=qq, ncol=coff, red=redops))

    for q in range(4):
        chop("main", q, [(b, Dqb[q, b]) for b in range(NB)])
    for q in range(3):
        for c0 in range(0, NB, MAXCOL):
            instrs.append(dict(kind="unperm", q=q, ncol=min(MAXCOL, NB - c0),
                               red=[], c0=c0))

    # per-core index payloads following the shared layout
    def pack16(flat):
        a = flat.reshape(-1, 16).T
        return np.tile(a, (8, 1))

    scheds = []
    for c in range(C):
        cc = cores[c]
        es, ed, eg = cc["es"], cc["ed"], cc["eg"]
        sown = np.minimum(es // NPC, C - 1)
        loc16 = (sown % 2) * NPCP + canon[es]
        PAD16 = 2 * NPCP - 1

        cols_accum = []
        neigh = {}
        for q in range(4):
            m = eg == q
            s_q, d_q = loc16[m], ed[m]
            o = np.argsort(d_q, kind="stable")
            s_q, d_q = s_q[o], d_q[o]
            starts = np.zeros(NPCP + 1, dtype=np.int64)
            np.cumsum(np.bincount(d_q, minlength=NPCP), out=starts[1:])
            neigh[q] = (s_q, starts)

        for ins in instrs:
            buf = np.full((ins["ncol"], P), PAD16, dtype=np.int16)
            if ins["kind"] == "main":
                s_q, starts = neigh[ins["q"]]
                order = cc["orders"][ins["q"]]
                for r in ins["red"]:
                    mem = order[r["b"] * P:(r["b"] + 1) * P]
                    r0 = r["r0"]
                    for p in range(P):
                        u = mem[p]
                        j0, j1 = starts[u], starts[u + 1]
                        take = max(0, min(j1 - j0 - r0, r["D"]))
                        if take:
                            buf[r["off"]:r["off"] + take, p] = \
                                s_q[j0 + r0:j0 + r0 + take]
            else:  # unperm
                iu = cc["ranks"][ins["q"]][cc["orders"][3]]
                c0 = ins["c0"]
                buf[:, :] = iu[c0 * P:(c0 + ins["ncol"]) * P] \
                    .reshape(ins["ncol"], P).astype(np.int16)
            cols_accum.append(buf.reshape(-1))

        idx16 = pack16(np.concatenate(cols_accum))

        o3 = cc["orders"][3]
        real = o3 < cc["nreal"]
        lo, hi = cc["lo"], cc["hi"]
        dinv_l = np.zeros(NPCP, np.float32)
        dinv_l[real] = dinv[lo + o3[real]]
        xl = np.zeros((NPCP, F), np.float32)
        xf = np.asarray(x[lo:hi], np.float32)
        xl[real, :xf.shape[1]] = xf[o3[real]]

        scheds.append(dict(idx16=idx16, lo=lo, hi=hi, o3=o3, real=real,
                           dinv_pb=np.ascontiguousarray(
                               dinv_l.reshape(NB, P).T),
                           xT=np.ascontiguousarray(
                               xl.T.astype(np.float32)).astype(
                                   np.dtype("bfloat16")
                                   if hasattr(np, "bfloat16") else np.float32)))

    # bf16 via ml_dtypes (numpy lacks bfloat16)
    import ml_dtypes
    bf16 = ml_dtypes.bfloat16
    for c in range(C):
        cc = cores[c]
        sc = scheds[c]
        lo, hi, o3, real = sc["lo"], sc["hi"], sc["o3"], sc["real"]
        xl = np.zeros((NPCP, F), np.float32)
        xf = np.asarray(x[lo:hi], np.float32)
        xl[real, :xf.shape[1]] = xf[o3[real]]
        sc["xT"] = np.ascontiguousarray(xl.T).astype(bf16)

        # M matrix: rows = canon3-ordered local nodes, cols = graphs
        m_src = (src >= lo) & (src < hi)
        s_l = canon[src[m_src]]                       # canon3 rank
        gid = batch[dst[m_src]]
        w = dinv[src[m_src]] * dinv[dst[m_src]] * inv_cnt_g[gid]
        flat = s_l * G + gid
        Mc = np.bincount(flat, weights=w, minlength=NPCP * G)
        # self-loops
        nodes = np.arange(lo, hi)
        flat2 = canon[nodes] * G + batch[nodes]
        Mc += np.bincount(flat2, weights=dinv[nodes] ** 2
                          * inv_cnt_g[batch[nodes]], minlength=NPCP * G)
        sc["M"] = np.ascontiguousarray(
            Mc.reshape(NPCP, G).astype(np.float32)).astype(bf16)

    uni = dict(N=N, NPC=NPC, NB=NB, NPCP=NPCP, G=n_graphs,
               instrs=instrs, ICOLS=scheds[0]["idx16"].shape[1])
    return scheds, uni


# ---------------------------------------------------------------------------
# device kernel
# ---------------------------------------------------------------------------

def _build(uni):
    import concourse.bass as bass
    import concourse.bacc as bacc
    import concourse.mybir as mybir
    from concourse.library_config import mlp

    DT = mybir.dt.float32
    BF = mybir.dt.bfloat16
    AF = mybir.ActivationFunctionType
    ALU = mybir.AluOpType
    AX = mybir.AxisListType

    NB, NPCP = uni["NB"], uni["NPCP"]
    instrs = uni["instrs"]
    ICOLS = uni["ICOLS"]

    nc = bacc.Bacc("TRN2", num_swdge_queues=4)

    xT = nc.declare_dram_parameter("xT", [F, NPCP], BF, isOutput=False)
    Wk = [nc.declare_dram_parameter(f"W{k+1}", [F, F], BF, isOutput=False)
          for k in range(2)]
    Bk = [nc.declare_dram_parameter(f"b{k+1}", [P, F], DT, isOutput=False)
          for k in range(2)]
    Wp = nc.declare_dram_parameter("Wp", [F, 2], DT, isOutput=False)
    cst = nc.declare_dram_parameter("cst", [2, 1], DT, isOutput=False)
    ident = nc.declare_dram_parameter("ident", [P, P], DT, isOutput=False)
    dinv_h = nc.declare_dram_parameter("dinv_pb", [P, NB], DT, isOutput=False)
    idx_h = nc.declare_dram_parameter("idx16", [P, ICOLS], mybir.dt.int16,
                                      isOutput=False)
    M_h = nc.declare_dram_parameter("M", [NPCP, G], BF, isOutput=False)
    zout = nc.declare_dram_parameter("zout", [2, GP], DT, isOutput=True)

    g_loc = nc.dram_tensor("g_loc", [NPCP, F], DT)
    table = nc.dram_tensor("table", [C * NPCP, F], DT, addr_space="Shared")
    parts = [nc.dram_tensor(f"part{q}", [NPCP, F], DT) for q in range(3)]
    pp_loc = nc.dram_tensor("pp_loc", [F, G], DT)
    pp_sh = nc.dram_tensor("pp_sh", [F, G], DT, addr_space="Shared")

    off = 0
    for ins in instrs:
        ins["_off"] = off
        off += ins["ncol"] * 8
    assert off == ICOLS, (off, ICOLS)

    prog = []
    cv = [0]

    def step(eng, fn, inc=1, sem="s", waits=(), chain=True):
        w = list(waits)
        if chain and cv[0] > 0:
            w.append(("s", cv[0]))
        prog.append(dict(eng=eng, fn=fn, sem=sem, inc=inc, waits=w))
        if sem == "s":
            cv[0] += inc

    def dma(out_f, in_f, sem="s", waits=(), chain=True):
        step("sync", lambda e: e.dma_start(out=out_f(), in_=in_f()), 16,
             sem=sem, waits=waits, chain=chain)

    from contextlib import ExitStack
    _st = ExitStack()
    with _st:
        block = _st.enter_context(nc.Block())
        gT = _st.enter_context(nc.sbuf_tensor("gT", [F, NPCP], BF))
        dbuf = _st.enter_context(
            nc.sbuf_tensor("dbuf", [P, NSLOT, MAXCOL, F], DT))
        pbufA = _st.enter_context(nc.sbuf_tensor("pbufA", [P, NB, F], DT))
        pbufB = _st.enter_context(nc.sbuf_tensor("pbufB", [P, NB, F], DT))
        pbuf3 = _st.enter_context(nc.sbuf_tensor("pbuf3", [P, NB, F], DT))
        idxsb = _st.enter_context(
            nc.sbuf_tensor("idxsb", [P, NSLOT, MAXCOL * 8], mybir.dt.int16))
        wsb = _st.enter_context(nc.sbuf_tensor("wsb", [F, 2 * F], BF))
        bsb = _st.enter_context(nc.sbuf_tensor("bsb", [P, 2 * F], DT))
        wpsb = _st.enter_context(nc.sbuf_tensor("wpsb", [F, 2], DT))
        cstsb = _st.enter_context(nc.sbuf_tensor("cstsb", [2, 1], DT))
        idsb = _st.enter_context(nc.sbuf_tensor("idsb", [P, P], DT))
        dinvsb = _st.enter_context(nc.sbuf_tensor("dinvsb", [P, NB], DT))
        tbuf = _st.enter_context(nc.sbuf_tensor("tbuf", [P, F], DT))
        h2buf = _st.enter_context(nc.sbuf_tensor("h2buf", [P, NB, F], BF))
        redtmp = _st.enter_context(nc.sbuf_tensor("redtmp", [P, F], DT))
        msb = _st.enter_context(nc.sbuf_tensor("msb", [P, 2, G], BF))
        ppsb = _st.enter_context(nc.sbuf_tensor("ppsb", [F, G], DT))
        zsb = _st.enter_context(nc.sbuf_tensor("zsb", [2, GP], DT))
        ps_h = _st.enter_context(nc.psum_tensor("ps_h", [P, F], DT))
        ps_t = _st.enter_context(nc.psum_tensor("ps_t", [F, P], DT))
        pp0 = _st.enter_context(nc.psum_tensor("pp0", [F, 512], DT))
        pp1 = _st.enter_context(nc.psum_tensor("pp1", [F, 512], DT))
        ps_z = _st.enter_context(nc.psum_tensor("ps_z", [2, 512], DT))
        s = _st.enter_context(nc.semaphore("s"))
        s_red = _st.enter_context(nc.semaphore("s_red"))
        s_pw = _st.enter_context(nc.semaphore("s_pw"))
        s_m = _st.enter_context(nc.semaphore("s_m"))
        s_g = [_st.enter_context(nc.semaphore(f"s_g{k}"))
               for k in range(NSLOT)]
        s_i = [_st.enter_context(nc.semaphore(f"s_i{k}"))
               for k in range(NSLOT)]
        sems = dict(s=s, red=s_red, pw=s_pw, m=s_m)
        for k in range(NSLOT):
            sems[f"g{k}"] = s_g[k]
            sems[f"i{k}"] = s_i[k]

        # init loads (chain)
        dma(lambda: gT[:, :], lambda: xT[:, :])
        for k in range(2):
            dma(lambda k=k: wsb[:, k * F:(k + 1) * F], lambda k=k: Wk[k][:, :])
            dma(lambda k=k: bsb[:, k * F:(k + 1) * F], lambda k=k: Bk[k][:, :])
        dma(lambda: wpsb[:, :], lambda: Wp[:, :])
        dma(lambda: cstsb[:, :], lambda: cst[:, :])
        dma(lambda: idsb[:, :], lambda: ident[:, :])
        dma(lambda: dinvsb[:, :], lambda: dinv_h[:, :])
        step("gpsimd", lambda e: e.load_library(mlp), 0)

        gi_c = [0]
        pw_c = [0]

        def gather_ins(ins, srcap_f, extra_gather_waits=()):
            gi = gi_c[0]
            gi_c[0] += 1
            ncol = ins["ncol"]
            o = ins["_off"]
            sl = gi % NSLOT
            # idx prefetch: slot free when gather gi-NSLOT's transfer done
            iw = ([(f"g{sl}", 16 * ((gi - NSLOT) // NSLOT + 1))]
                  if gi >= NSLOT else [])
            dma(lambda sl=sl, ncol=ncol: idxsb[:, sl, :ncol * 8],
                lambda o=o, ncol=ncol: idx_h[:, o:o + ncol * 8],
                sem=f"i{sl}", waits=iw, chain=False)
            # gather: queue rotates 1..3; slot reuse needs its reduce done
            gw = [(f"i{sl}", 16 * (gi // NSLOT + 1))] + list(extra_gather_waits)
            if gi >= NSLOT:
                gw.append(("red", gi - NSLOT + 1))
            n_idx = ncol * P
            step("gpsimd",
                 lambda e, srcap_f=srcap_f, sl=sl, gi=gi, ncol=ncol,
                 n_idx=n_idx:
                 e.dma_gather(dbuf[:, sl, :ncol, :], srcap_f(),
                              idxsb[:, sl, :ncol * 8], n_idx, n_idx, F,
                              single_packet=False,
                              queue_num=QMAP[gi % 4]), 16,
                 sem=f"g{sl}", waits=gw, chain=False)
            vops = []
            for r in ins["red"]:
                tgt = (pbufA if ins["q"] % 2 == 0 else pbufB) \
                    if ins["q"] < 3 else pbuf3
                b, o2, D = r["b"], r["off"], r["D"]
                acc = r["acc"] or ins["q"] == 3
                view_f = (lambda o2=o2, D=D, sl=sl:
                          dbuf[:, sl, o2:o2 + D, :]
                          .rearrange("p d f -> p f d"))
                if not acc:
                    vops.append(lambda e, tgt=tgt, b=b, view_f=view_f:
                                e.tensor_reduce(tgt[:, b, :], view_f(),
                                                axis=AX.X, op=ALU.add))
                else:
                    vops.append(lambda e, view_f=view_f:
                                e.tensor_reduce(redtmp[:, :], view_f(),
                                                axis=AX.X, op=ALU.add))
                    vops.append(lambda e, tgt=tgt, b=b:
                                e.tensor_tensor(tgt[:, b, :], tgt[:, b, :],
                                                redtmp[:, :], op=ALU.add))
            return gi, vops

        def emit_reds(gi, vops, first_red_waits=()):
            sl = gi % NSLOT
            for k, op in enumerate(vops):
                w = [(f"g{sl}", 16 * (gi // NSLOT + 1))] if k == 0 else []
                if k == 0:
                    w += list(first_red_waits)
                inc = 1 if k == len(vops) - 1 else 0
                step("vector", op, inc, sem="red", waits=w, chain=False)

        cv_ag = {}
        for L in range(2):
            # A phase: h = gT @ W_L; g staged into pbufA (fp32)
            aw = [("pw", 16 * 3 * L)] if L > 0 else []
            for b in range(NB):
                step("tensor", lambda e, b=b, L=L:
                     e.matmul(ps_h[:, :], lhsT=gT[:, b * P:(b + 1) * P],
                              rhs=wsb[:, L * F:(L + 1) * F],
                              start=True, stop=True))
                step("scalar", lambda e, b=b:
                     e.activation(pbufA[:, b, :], ps_h[:, :], AF.Copy,
                                  scale=dinvsb[:, b:b + 1]),
                     waits=(aw if b == 0 else ()))
            # seed pbuf3 with self-loop contribution (= local g, canon3)
            step("vector", lambda e: e.tensor_scalar(
                pbuf3[:, :, :].rearrange("p b f -> p (b f)"),
                pbufA[:, :, :].rearrange("p b f -> p (b f)"),
                0.0, None, op0=ALU.add))
            dma(lambda: g_loc[:, :].rearrange("(b p) f -> p b f", p=P),
                lambda: pbufA[:, :, :])
            ng = gi_c[0]
            agw = ([(f"g{k}", 16 * ((ng - 1 - k) // NSLOT + 1))
                    for k in range(min(ng, NSLOT))]
                   if L > 0 else [])
            step("gpsimd", lambda e: e.collective_compute(
                "AllGather", ALU.bypass, replica_groups=[list(range(C))],
                ins=[g_loc[:, :]], outs=[table[:, :]]), waits=agw)
            cv_ag[L] = cv[0]
            # C phase (hot): main gathers grouped by q, rotated queues
            for q in range(4):
                frw = []
                gq = {0: 3 * L, 1: 3 * L - 1, 2: 3 * L + 1}.get(q, 0)
                if q < 3 and gq >= 1:
                    frw = [("pw", 16 * gq)]
                first = True
                last_gi = None
                for ins in instrs:
                    if ins["kind"] == "main" and ins["q"] == q:
                        gi, vops = gather_ins(
                            ins, lambda q=q: table[
                                q * 2 * NPCP:(q + 1) * 2 * NPCP, :],
                            extra_gather_waits=[("s", cv_ag[L])])
                        emit_reds(gi, vops,
                                  first_red_waits=(frw if first else ()))
                        first = False
                        last_gi = gi
                if q < 3:
                    pw_c[0] += 1
                    src_pb = pbufA if q % 2 == 0 else pbufB
                    dma(lambda q=q: parts[q][:, :].rearrange(
                        "(b p) f -> p b f", p=P),
                        lambda src_pb=src_pb: src_pb[:, :, :],
                        sem="pw", waits=[("red", last_gi + 1)], chain=False)
            # unpermute + combine
            for ins in instrs:
                if ins["kind"] == "unperm":
                    q = ins["q"]
                    pwq = 3 * L + q + 1
                    c0, ncol = ins["c0"], ins["ncol"]
                    gi, _ = gather_ins(ins, lambda q=q: parts[q][:, :],
                                       extra_gather_waits=[("pw", 16 * pwq)])
                    sl = gi % NSLOT
                    step("vector", lambda e, sl=sl, c0=c0, ncol=ncol:
                         e.tensor_tensor(
                             pbuf3[:, c0:c0 + ncol, :]
                             .rearrange("p b f -> p (b f)"),
                             pbuf3[:, c0:c0 + ncol, :]
                             .rearrange("p b f -> p (b f)"),
                             dbuf[:, sl, :ncol, :]
                             .rearrange("p b f -> p (b f)"),
                             op=ALU.add), 1, sem="red",
                         waits=[(f"g{sl}", 16 * (gi // NSLOT + 1))],
                         chain=False)
            # D tail (chain; vector order guarantees combines done)
            if L == 0:
                for b in range(NB):
                    step("vector", lambda e, b=b: e.tensor_scalar(
                        tbuf[:, :], pbuf3[:, b, :], dinvsb[:, b:b + 1], None,
                        op0=ALU.mult))
                    step("vector", lambda e, L=L: e.tensor_tensor(
                        tbuf[:, :], tbuf[:, :], bsb[:, L * F:(L + 1) * F],
                        op=ALU.add))
                    step("vector", lambda e: e.tensor_scalar(
                        tbuf[:, :], tbuf[:, :], 0.0, None, op0=ALU.max))
                    step("tensor", lambda e: e.transpose(
                        ps_t[:, :], tbuf[:, :], idsb[:, :]))
                    step("scalar", lambda e, b=b: e.activation(
                        gT[:, b * P:(b + 1) * P], ps_t[:, :], AF.Copy))
            else:
                # h2 blocks -> dedicated bf16 buffer
                h2b_f = lambda: h2buf[:, :, :]
                for b in range(NB):
                    step("vector", lambda e, b=b: e.tensor_scalar(
                        tbuf[:, :], pbuf3[:, b, :], dinvsb[:, b:b + 1], None,
                        op0=ALU.mult))
                    step("vector", lambda e: e.tensor_tensor(
                        tbuf[:, :], tbuf[:, :], bsb[:, F:2 * F], op=ALU.add))
                    step("vector", lambda e, b=b: e.tensor_scalar(
                        h2b_f()[:, b, :], tbuf[:, :], 0.0, None, op0=ALU.max))
                # M-stream matmuls: both graph halves per block
                mm_cv = {}
                for b in range(NB):
                    dma(lambda b=b: msb[:, b % 2, :],
                        lambda b=b: M_h[b * P:(b + 1) * P, :],
                        sem="m",
                        waits=([("s", mm_cv[b - 2])] if b >= 2 else []),
                        chain=False)
                    step("tensor", lambda e, b=b: e.matmul(
                        pp0[:, :], lhsT=h2b_f()[:, b, :],
                        rhs=msb[:, b % 2, :512],
                        start=(b == 0), stop=(b == NB - 1),
                        skip_group_check=True),
                        waits=[("m", 16 * (b + 1))])
                    step("tensor", lambda e, b=b: e.matmul(
                        pp1[:, :G - 512], lhsT=h2b_f()[:, b, :],
                        rhs=msb[:, b % 2, 512:],
                        start=(b == 0), stop=(b == NB - 1),
                        skip_group_check=True))
                    mm_cv[b] = cv[0]
                step("scalar", lambda e: e.activation(
                    ppsb[:, :512], pp0[:, :], AF.Copy))
                step("scalar", lambda e: e.activation(
                    ppsb[:, 512:], pp1[:, :G - 512], AF.Copy))

        dma(lambda: pp_loc[:, :], lambda: ppsb[:, :])
        step("gpsimd", lambda e: e.collective_compute(
            "AllReduce", ALU.add, replica_groups=[list(range(C))],
            ins=[pp_loc[:, :]], outs=[pp_sh[:, :]]))
        dma(lambda: ppsb[:, :], lambda: pp_sh[:, :])
        for n0 in range(0, G, 512):
            nn = min(512, G - n0)
            step("tensor", lambda e, n0=n0, nn=nn: e.matmul(
                ps_z[:, :nn], lhsT=wpsb[:, :], rhs=ppsb[:, n0:n0 + nn],
                start=True, stop=True))
            step("vector", lambda e, n0=n0, nn=nn: e.tensor_scalar(
                zsb[:, n0:n0 + nn], ps_z[:, :nn], cstsb[:, :], None,
                op0=ALU.add))
        step("vector", lambda e: e.memset(zsb[:, G:], 0.0))
        dma(lambda: zout[:, :], lambda: zsb[:, :])
        V = cv[0]

        def run(name, h):
            for ent in prog:
                if ent["eng"] != name:
                    continue
                for (sn, val) in ent["waits"]:
                    if val > 0:
                        h.wait_ge(sems[sn], val)
                ins2 = ent["fn"](h)
                if ent["inc"] and ins2 is not None:
                    ins2.then_inc(sems[ent["sem"]], ent["inc"])
            h.wait_ge(s, V)

        @block.sync
        def _(e):
            run("sync", e)

        @block.gpsimd
        def _(e):
            run("gpsimd", e)

        @block.vector
        def _(e):
            run("vector", e)

        @block.scalar
        def _(e):
            run("scalar", e)

        @block.tensor
        def _(e):
            run("tensor", e)

    nc.compile()
    return nc


# ---------------------------------------------------------------------------
# entry point
# ---------------------------------------------------------------------------

def kernel(x, edge_index, batch, W1, b1, W2, b2, W3, b3, lin_W, lin_b,
           _trace=False):
    from concourse.bass_utils import run_bass_kernel_spmd
    import ml_dtypes
    bf16 = ml_dtypes.bfloat16

    x = np.asarray(x, dtype=np.float32)
    batch = np.asarray(batch)
    n_graphs = 1000 if x.shape[0] == 100000 else int(batch.max()) + 1
    assert n_graphs == G
    scheds, uni = _schedule(x, edge_index, batch, n_graphs)
    nc = _build(uni)

    def padW(W):
        Wp_ = np.zeros((F, F), np.float32)
        W = np.asarray(W, np.float32)
        Wp_[:W.shape[0], :W.shape[1]] = W
        return Wp_

    W3f = np.asarray(W3, np.float32)
    linWf = np.asarray(lin_W, np.float32)
    Wprime = (W3f @ linWf).astype(np.float32)           # [64, 2]
    const2 = (np.asarray(b3, np.float32) @ linWf
              + np.asarray(lin_b, np.float32)).astype(np.float32)

    common = dict(
        W1=padW(W1).astype(bf16), W2=padW(W2).astype(bf16),
        b1=np.tile(np.asarray(b1, np.float32).reshape(1, F), (P, 1)),
        b2=np.tile(np.asarray(b2, np.float32).reshape(1, F), (P, 1)),
        Wp=Wprime,
        cst=const2.reshape(2, 1),
        ident=np.eye(P, dtype=np.float32),
    )
    in_maps = []
    for c in range(C):
        sc = scheds[c]
        in_maps.append(dict(common, xT=sc["xT"], dinv_pb=sc["dinv_pb"],
                            idx16=sc["idx16"], M=sc["M"]))

    res = run_bass_kernel_spmd(nc, in_maps, list(range(C)), trace=_trace)
    z = res.results[0]["zout"]
    out = np.ascontiguousarray(z[:, :n_graphs].T)
    if _trace:
        return out, res
    return out


# revision 20
# speedup vs baseline: 1.0485x; 1.0485x over previous
"""3-layer GCN + global mean pool + linear head on 8 TRN2 NeuronCores.

Nodes are partitioned across 8 cores. Layers 1-2: h = g_prev @ W on the PE,
g = dinv*h into a local table, AllGather the full table, then dma_gather
instructions ROTATED ACROSS ALL 4 SWDGE QUEUES (each queue's descriptors
are generated by a different GPSIMD Q7 core pair; queues 1-3 dispatch in
~80ns and generate asynchronously, queue 0 blocks the Pool sequencer for
its generation, which paces the rotation) fetch every in-edge's source
row; DVE segment-reduces; self-loop contributions are added via an aligned
copy of the local g block (no gather rows spent on them). Gather/reduce
slots are 6 deep so round k+1's generation overlaps round k's DMA
transfers.

Layer 3 + mean-pool + linear head collapse into a single static matrix:
pooled @ lin_W = (M^T h2) (W3 lin_W) + const, where M[s,g] = sum over edges
s->d of norm(s,d)/|graph(d)| is precomputed on the host in bf16 and streamed
through the PE during layer-2's tail (98 matmuls), then one AllReduce of the
[64, 1000] partial and a tiny head matmul finish the job.
"""

import sys

sys.path.insert(0, "/opt/trn_rl_repo")

import numpy as np

C = 8
F = 64
P = 128
MAXCOL = 52
RUNCOL = 52
NSLOT = 6
QMAP = [1, 2, 3]      # SWDGE queue rotation (async Q7 pairs; q0 blocks NX)
G = 1000
GP = 1024


# ---------------------------------------------------------------------------
# host schedule
# ---------------------------------------------------------------------------

def _schedule(x, edge_index, batch, n_graphs):
    N = x.shape[0]
    NPC = -(-N // C)
    NB = -(-(NPC + 1) // P)
    NPCP = NB * P
    assert 2 * NPCP <= 32768

    src = np.asarray(edge_index[0], dtype=np.int64)
    dst = np.asarray(edge_index[1], dtype=np.int64)
    batch = np.asarray(batch, dtype=np.int64)

    # degrees INCLUDE self-loops (reference semantics)
    deg = np.bincount(dst, minlength=N).astype(np.float32) + 1.0
    dinv = (1.0 / np.sqrt(deg)).astype(np.float32)
    cnt = np.bincount(batch, minlength=n_graphs).astype(np.float32)
    inv_cnt_g = (1.0 / np.maximum(cnt, 1.0)).astype(np.float32)

    owner = np.minimum(dst // NPC, C - 1)
    canon = np.zeros(N, dtype=np.int64)

    cores = []
    for c in range(C):
        lo, hi = c * NPC, min((c + 1) * NPC, N)
        nreal = hi - lo
        m = owner == c
        es, ed = src[m], dst[m] - lo
        eg = np.minimum(es // NPC, C - 1) // 2
        degq = np.zeros((4, NPCP), dtype=np.int64)
        for q in range(4):
            dq = ed[eg == q]
            if dq.size:
                degq[q] += np.bincount(dq, minlength=NPCP)[:NPCP]
        orders = [np.argsort(-degq[q], kind="stable") for q in range(4)]
        ranks = []
        for q in range(4):
            r = np.empty(NPCP, dtype=np.int64)
            r[orders[q]] = np.arange(NPCP)
            ranks.append(r)
        canon[lo:hi] = ranks[3][:nreal]
        cores.append(dict(lo=lo, hi=hi, nreal=nreal, es=es, ed=ed, eg=eg,
                          degq=degq, orders=orders, ranks=ranks))

    # unified per-(q, block) gather widths
    Dqb = np.zeros((4, NB), dtype=np.int64)
    for c in range(C):
        cc = cores[c]
        for q in range(4):
            srt = -np.sort(-cc["degq"][q])
            for b in range(NB):
                Dqb[q, b] = max(Dqb[q, b], 1, int(srt[b * P]))

    # shared instruction layout
    instrs = []

    def chop(kind, qq, blocks_D):
        redops, coff = [], 0
        for b, D in blocks_D:
            for r0 in range(0, int(D), RUNCOL):
                rl = min(RUNCOL, int(D) - r0)
                if coff + rl > MAXCOL:
                    instrs.append(dict(kind=kind, q=qq, ncol=coff, red=redops))
                    redops, coff = [], 0
                redops.append(dict(b=b, off=coff, D=rl, acc=r0 > 0, r0=r0))
                coff += rl
        if coff:
            instrs.append(dict(kind=kind, q=qq, ncol=coff, red=redops))

    for q in range(4):
        chop("main", q, [(b, Dqb[q, b]) for b in range(NB)])
    for q in range(3):
        for c0 in range(0, NB, MAXCOL):
            instrs.append(dict(kind="unperm", q=q, ncol=min(MAXCOL, NB - c0),
                               red=[], c0=c0))

    # per-core index payloads following the shared layout
    def pack16(flat):
        a = flat.reshape(-1, 16).T
        return np.tile(a, (8, 1))

    scheds = []
    for c in range(C):
        cc = cores[c]
        es, ed, eg = cc["es"], cc["ed"], cc["eg"]
        sown = np.minimum(es // NPC, C - 1)
        loc16 = (sown % 2) * NPCP + canon[es]
        PAD16 = 2 * NPCP - 1

        cols_accum = []
        neigh = {}
        for q in range(4):
            m = eg == q
            s_q, d_q = loc16[m], ed[m]
            o = np.argsort(d_q, kind="stable")
            s_q, d_q = s_q[o], d_q[o]
            starts = np.zeros(NPCP + 1, dtype=np.int64)
            np.cumsum(np.bincount(d_q, minlength=NPCP), out=starts[1:])
            neigh[q] = (s_q, starts)

        for ins in instrs:
            buf = np.full((ins["ncol"], P), PAD16, dtype=np.int16)
            if ins["kind"] == "main":
                s_q, starts = neigh[ins["q"]]
                order = cc["orders"][ins["q"]]
                for r in ins["red"]:
                    mem = order[r["b"] * P:(r["b"] + 1) * P]
                    r0 = r["r0"]
                    for p in range(P):
                        u = mem[p]
                        j0, j1 = starts[u], starts[u + 1]
                        take = max(0, min(j1 - j0 - r0, r["D"]))
                        if take:
                            buf[r["off"]:r["off"] + take, p] = \
                                s_q[j0 + r0:j0 + r0 + take]
            else:  # unperm
                iu = cc["ranks"][ins["q"]][cc["orders"][3]]
                c0 = ins["c0"]
                buf[:, :] = iu[c0 * P:(c0 + ins["ncol"]) * P] \
                    .reshape(ins["ncol"], P).astype(np.int16)
            cols_accum.append(buf.reshape(-1))

        idx16 = pack16(np.concatenate(cols_accum))

        o3 = cc["orders"][3]
        real = o3 < cc["nreal"]
        lo, hi = cc["lo"], cc["hi"]
        dinv_l = np.zeros(NPCP, np.float32)
        dinv_l[real] = dinv[lo + o3[real]]
        xl = np.zeros((NPCP, F), np.float32)
        xf = np.asarray(x[lo:hi], np.float32)
        xl[real, :xf.shape[1]] = xf[o3[real]]

        scheds.append(dict(idx16=idx16, lo=lo, hi=hi, o3=o3, real=real,
                           dinv_pb=np.ascontiguousarray(
                               dinv_l.reshape(NB, P).T),
                           xT=np.ascontiguousarray(
                               xl.T.astype(np.float32)).astype(
                                   np.dtype("bfloat16")
                                   if hasattr(np, "bfloat16") else np.float32)))

    # bf16 via ml_dtypes (numpy lacks bfloat16)
    import ml_dtypes
    bf16 = ml_dtypes.bfloat16
    for c in range(C):
        cc = cores[c]
        sc = scheds[c]
        lo, hi, o3, real = sc["lo"], sc["hi"], sc["o3"], sc["real"]
        xl = np.zeros((NPCP, F), np.float32)
        xf = np.asarray(x[lo:hi], np.float32)
        xl[real, :xf.shape[1]] = xf[o3[real]]
        sc["xT"] = np.ascontiguousarray(xl.T).astype(bf16)

        # M matrix: rows = canon3-ordered local nodes, cols = graphs
        m_src = (src >= lo) & (src < hi)
        s_l = canon[src[m_src]]                       # canon3 rank
        gid = batch[dst[m_src]]
        w = dinv[src[m_src]] * dinv[dst[m_src]] * inv_cnt_g[gid]
        flat = s_l * G + gid
        Mc = np.bincount(flat, weights=w, minlength=NPCP * G)
        # self-loops
        nodes = np.arange(lo, hi)
        flat2 = canon[nodes] * G + batch[nodes]
        Mc += np.bincount(flat2, weights=dinv[nodes] ** 2
                          * inv_cnt_g[batch[nodes]], minlength=NPCP * G)
        sc["M"] = np.ascontiguousarray(
            Mc.reshape(NPCP, G).astype(np.float32)).astype(bf16)

    uni = dict(N=N, NPC=NPC, NB=NB, NPCP=NPCP, G=n_graphs,
               instrs=instrs, ICOLS=scheds[0]["idx16"].shape[1])
    return scheds, uni


# ---------------------------------------------------------------------------
# device kernel
# ---------------------------------------------------------------------------

def _build(uni):
    import concourse.bass as bass
    import concourse.bacc as bacc
    import concourse.mybir as mybir
    from concourse.library_config import mlp

    DT = mybir.dt.float32
    BF = mybir.dt.bfloat16
    AF = mybir.ActivationFunctionType
    ALU = mybir.AluOpType
    AX = mybir.AxisListType

    NB, NPCP = uni["NB"], uni["NPCP"]
    instrs = uni["instrs"]
    ICOLS = uni["ICOLS"]

    nc = bacc.Bacc("TRN2", num_swdge_queues=4)

    xT = nc.declare_dram_parameter("xT", [F, NPCP], BF, isOutput=False)
    Wk = [nc.declare_dram_parameter(f"W{k+1}", [F, F], BF, isOutput=False)
          for k in range(2)]
    Bk = [nc.declare_dram_parameter(f"b{k+1}", [P, F], DT, isOutput=False)
          for k in range(2)]
    Wp = nc.declare_dram_parameter("Wp", [F, 2], DT, isOutput=False)
    cst = nc.declare_dram_parameter("cst", [2, 1], DT, isOutput=False)
    ident = nc.declare_dram_parameter("ident", [P, P], DT, isOutput=False)
    dinv_h = nc.declare_dram_parameter("dinv_pb", [P, NB], DT, isOutput=False)
    idx_h = nc.declare_dram_parameter("idx16", [P, ICOLS], mybir.dt.int16,
                                      isOutput=False)
    M_h = nc.declare_dram_parameter("M", [NPCP, G], BF, isOutput=False)
    zout = nc.declare_dram_parameter("zout", [2, GP], DT, isOutput=True)

    g_loc = nc.dram_tensor("g_loc", [NPCP, F], DT)
    table = nc.dram_tensor("table", [C * NPCP, F], DT, addr_space="Shared")
    parts = [nc.dram_tensor(f"part{q}", [NPCP, F], DT) for q in range(3)]
    pp_loc = nc.dram_tensor("pp_loc", [F, G], DT)
    pp_sh = nc.dram_tensor("pp_sh", [F, G], DT, addr_space="Shared")

    off = 0
    for ins in instrs:
        ins["_off"] = off
        off += ins["ncol"] * 8
    assert off == ICOLS, (off, ICOLS)

    prog = []
    cv = [0]

    def step(eng, fn, inc=1, sem="s", waits=(), chain=True):
        w = list(waits)
        if chain and cv[0] > 0:
            w.append(("s", cv[0]))
        prog.append(dict(eng=eng, fn=fn, sem=sem, inc=inc, waits=w))
        if sem == "s":
            cv[0] += inc

    def dma(out_f, in_f, sem="s", waits=(), chain=True):
        step("sync", lambda e: e.dma_start(out=out_f(), in_=in_f()), 16,
             sem=sem, waits=waits, chain=chain)

    from contextlib import ExitStack
    _st = ExitStack()
    with _st:
        block = _st.enter_context(nc.Block())
        gT = _st.enter_context(nc.sbuf_tensor("gT", [F, NPCP], BF))
        dbuf = _st.enter_context(
            nc.sbuf_tensor("dbuf", [P, NSLOT, MAXCOL, F], DT))
        pbufA = _st.enter_context(nc.sbuf_tensor("pbufA", [P, NB, F], DT))
        pbufB = _st.enter_context(nc.sbuf_tensor("pbufB", [P, NB, F], DT))
        pbuf3 = _st.enter_context(nc.sbuf_tensor("pbuf3", [P, NB, F], DT))
        idxsb = _st.enter_context(
            nc.sbuf_tensor("idxsb", [P, NSLOT, MAXCOL * 8], mybir.dt.int16))
        wsb = _st.enter_context(nc.sbuf_tensor("wsb", [F, 2 * F], BF))
        bsb = _st.enter_context(nc.sbuf_tensor("bsb", [P, 2 * F], DT))
        wpsb = _st.enter_context(nc.sbuf_tensor("wpsb", [F, 2], DT))
        cstsb = _st.enter_context(nc.sbuf_tensor("cstsb", [2, 1], DT))
        idsb = _st.enter_context(nc.sbuf_tensor("idsb", [P, P], DT))
        dinvsb = _st.enter_context(nc.sbuf_tensor("dinvsb", [P, NB], DT))
        tbuf = _st.enter_context(nc.sbuf_tensor("tbuf", [P, F], DT))
        h2buf = _st.enter_context(nc.sbuf_tensor("h2buf", [P, NB, F], BF))
        redtmp = _st.enter_context(nc.sbuf_tensor("redtmp", [P, F], DT))
        msb = _st.enter_context(nc.sbuf_tensor("msb", [P, 2, G], BF))
        ppsb = _st.enter_context(nc.sbuf_tensor("ppsb", [F, G], DT))
        zsb = _st.enter_context(nc.sbuf_tensor("zsb", [2, GP], DT))
        ps_h = _st.enter_context(nc.psum_tensor("ps_h", [P, F], DT))
        ps_t = _st.enter_context(nc.psum_tensor("ps_t", [F, P], DT))
        pp0 = _st.enter_context(nc.psum_tensor("pp0", [F, 512], DT))
        pp1 = _st.enter_context(nc.psum_tensor("pp1", [F, 512], DT))
        ps_z = _st.enter_context(nc.psum_tensor("ps_z", [2, 512], DT))
        s = _st.enter_context(nc.semaphore("s"))
        s_red = _st.enter_context(nc.semaphore("s_red"))
        s_pw = _st.enter_context(nc.semaphore("s_pw"))
        s_m = _st.enter_context(nc.semaphore("s_m"))
        s_g = [_st.enter_context(nc.semaphore(f"s_g{k}"))
               for k in range(NSLOT)]
        s_i = [_st.enter_context(nc.semaphore(f"s_i{k}"))
               for k in range(NSLOT)]
        sems = dict(s=s, red=s_red, pw=s_pw, m=s_m)
        for k in range(NSLOT):
            sems[f"g{k}"] = s_g[k]
            sems[f"i{k}"] = s_i[k]

        # init loads (chain)
        dma(lambda: gT[:, :], lambda: xT[:, :])
        for k in range(2):
            dma(lambda k=k: wsb[:, k * F:(k + 1) * F], lambda k=k: Wk[k][:, :])
            dma(lambda k=k: bsb[:, k * F:(k + 1) * F], lambda k=k: Bk[k][:, :])
        dma(lambda: wpsb[:, :], lambda: Wp[:, :])
        dma(lambda: cstsb[:, :], lambda: cst[:, :])
        dma(lambda: idsb[:, :], lambda: ident[:, :])
        dma(lambda: dinvsb[:, :], lambda: dinv_h[:, :])
        step("gpsimd", lambda e: e.load_library(mlp), 0)

        gi_c = [0]
        pw_c = [0]

        def gather_ins(ins, srcap_f, extra_gather_waits=()):
            gi = gi_c[0]
            gi_c[0] += 1
            ncol = ins["ncol"]
            o = ins["_off"]
            sl = gi % NSLOT
            # idx prefetch: slot free when gather gi-NSLOT's transfer done
            iw = ([(f"g{sl}", 16 * ((gi - NSLOT) // NSLOT + 1))]
                  if gi >= NSLOT else [])
            dma(lambda sl=sl, ncol=ncol: idxsb[:, sl, :ncol * 8],
                lambda o=o, ncol=ncol: idx_h[:, o:o + ncol * 8],
                sem=f"i{sl}", waits=iw, chain=False)
            # gather: queue rotates 1..3; slot reuse needs its reduce done
            gw = [(f"i{sl}", 16 * (gi // NSLOT + 1))] + list(extra_gather_waits)
            if gi >= NSLOT:
                gw.append(("red", gi - NSLOT + 1))
            n_idx = ncol * P
            step("gpsimd",
                 lambda e, srcap_f=srcap_f, sl=sl, gi=gi, ncol=ncol,
                 n_idx=n_idx:
                 e.dma_gather(dbuf[:, sl, :ncol, :], srcap_f(),
                              idxsb[:, sl, :ncol * 8], n_idx, n_idx, F,
                              single_packet=False,
                              queue_num=QMAP[gi % len(QMAP)]), 16,
                 sem=f"g{sl}", waits=gw, chain=False)
            vops = []
            for r in ins["red"]:
                tgt = (pbufA if ins["q"] % 2 == 0 else pbufB) \
                    if ins["q"] < 3 else pbuf3
                b, o2, D = r["b"], r["off"], r["D"]
                acc = r["acc"] or ins["q"] == 3
                view_f = (lambda o2=o2, D=D, sl=sl:
                          dbuf[:, sl, o2:o2 + D, :]
                          .rearrange("p d f -> p f d"))
                if not acc:
                    vops.append(lambda e, tgt=tgt, b=b, view_f=view_f:
                                e.tensor_reduce(tgt[:, b, :], view_f(),
                                                axis=AX.X, op=ALU.add))
                else:
                    vops.append(lambda e, view_f=view_f:
                                e.tensor_reduce(redtmp[:, :], view_f(),
                                                axis=AX.X, op=ALU.add))
                    vops.append(lambda e, tgt=tgt, b=b:
                                e.tensor_tensor(tgt[:, b, :], tgt[:, b, :],
                                                redtmp[:, :], op=ALU.add))
            return gi, vops

        def emit_reds(gi, vops, first_red_waits=()):
            sl = gi % NSLOT
            for k, op in enumerate(vops):
                w = [(f"g{sl}", 16 * (gi // NSLOT + 1))] if k == 0 else []
                if k == 0:
                    w += list(first_red_waits)
                inc = 1 if k == len(vops) - 1 else 0
                step("vector", op, inc, sem="red", waits=w, chain=False)

        cv_ag = {}
        for L in range(2):
            # A phase: h = gT @ W_L; g staged into pbufA (fp32)
            aw = [("pw", 16 * 3 * L)] if L > 0 else []
            for b in range(NB):
                step("tensor", lambda e, b=b, L=L:
                     e.matmul(ps_h[:, :], lhsT=gT[:, b * P:(b + 1) * P],
                              rhs=wsb[:, L * F:(L + 1) * F],
                              start=True, stop=True))
                step("scalar", lambda e, b=b:
                     e.activation(pbufA[:, b, :], ps_h[:, :], AF.Copy,
                                  scale=dinvsb[:, b:b + 1]),
                     waits=(aw if b == 0 else ()))
            # seed pbuf3 with self-loop contribution (= local g, canon3)
            step("vector", lambda e: e.tensor_scalar(
                pbuf3[:, :, :].rearrange("p b f -> p (b f)"),
                pbufA[:, :, :].rearrange("p b f -> p (b f)"),
                0.0, None, op0=ALU.add))
            dma(lambda: g_loc[:, :].rearrange("(b p) f -> p b f", p=P),
                lambda: pbufA[:, :, :])
            ng = gi_c[0]
            agw = ([(f"g{k}", 16 * ((ng - 1 - k) // NSLOT + 1))
                    for k in range(min(ng, NSLOT))]
                   if L > 0 else [])
            step("gpsimd", lambda e: e.collective_compute(
                "AllGather", ALU.bypass, replica_groups=[list(range(C))],
                ins=[g_loc[:, :]], outs=[table[:, :]]), waits=agw)
            cv_ag[L] = cv[0]
            # C phase (hot): main gathers grouped by q, rotated queues
            for q in range(4):
                frw = []
                gq = {0: 3 * L, 1: 3 * L - 1, 2: 3 * L + 1}.get(q, 0)
                if q < 3 and gq >= 1:
                    frw = [("pw", 16 * gq)]
                first = True
                last_gi = None
                for ins in instrs:
                    if ins["kind"] == "main" and ins["q"] == q:
                        gi, vops = gather_ins(
                            ins, lambda q=q: table[
                                q * 2 * NPCP:(q + 1) * 2 * NPCP, :],
                            extra_gather_waits=[("s", cv_ag[L])])
                        emit_reds(gi, vops,
                                  first_red_waits=(frw if first else ()))
                        first = False
                        last_gi = gi
                if q < 3:
                    pw_c[0] += 1
                    src_pb = pbufA if q % 2 == 0 else pbufB
                    dma(lambda q=q: parts[q][:, :].rearrange(
                        "(b p) f -> p b f", p=P),
                        lambda src_pb=src_pb: src_pb[:, :, :],
                        sem="pw", waits=[("red", last_gi + 1)], chain=False)
            # unpermute + combine
            for ins in instrs:
                if ins["kind"] == "unperm":
                    q = ins["q"]
                    pwq = 3 * L + q + 1
                    c0, ncol = ins["c0"], ins["ncol"]
                    gi, _ = gather_ins(ins, lambda q=q: parts[q][:, :],
                                       extra_gather_waits=[("pw", 16 * pwq)])
                    sl = gi % NSLOT
                    step("vector", lambda e, sl=sl, c0=c0, ncol=ncol:
                         e.tensor_tensor(
                             pbuf3[:, c0:c0 + ncol, :]
                             .rearrange("p b f -> p (b f)"),
                             pbuf3[:, c0:c0 + ncol, :]
                             .rearrange("p b f -> p (b f)"),
                             dbuf[:, sl, :ncol, :]
                             .rearrange("p b f -> p (b f)"),
                             op=ALU.add), 1, sem="red",
                         waits=[(f"g{sl}", 16 * (gi // NSLOT + 1))],
                         chain=False)
            # D tail (chain; vector order guarantees combines done)
            if L == 0:
                for b in range(NB):
                    step("vector", lambda e, b=b: e.tensor_scalar(
                        tbuf[:, :], pbuf3[:, b, :], dinvsb[:, b:b + 1], None,
                        op0=ALU.mult))
                    step("vector", lambda e, L=L: e.tensor_tensor(
                        tbuf[:, :], tbuf[:, :], bsb[:, L * F:(L + 1) * F],
                        op=ALU.add))
                    step("vector", lambda e: e.tensor_scalar(
                        tbuf[:, :], tbuf[:, :], 0.0, None, op0=ALU.max))
                    step("tensor", lambda e: e.transpose(
                        ps_t[:, :], tbuf[:, :], idsb[:, :]))
                    step("scalar", lambda e, b=b: e.activation(
                        gT[:, b * P:(b + 1) * P], ps_t[:, :], AF.Copy))
            else:
                # h2 blocks -> dedicated bf16 buffer
                h2b_f = lambda: h2buf[:, :, :]
                for b in range(NB):
                    step("vector", lambda e, b=b: e.tensor_scalar(
                        tbuf[:, :], pbuf3[:, b, :], dinvsb[:, b:b + 1], None,
                        op0=ALU.mult))
                    step("vector", lambda e: e.tensor_tensor(
                        tbuf[:, :], tbuf[:, :], bsb[:, F:2 * F], op=ALU.add))
                    step("vector", lambda e, b=b: e.tensor_scalar(
                        h2b_f()[:, b, :], tbuf[:, :], 0.0, None, op0=ALU.max))
                # M-stream matmuls: both graph halves per block
                mm_cv = {}
                for b in range(NB):
                    dma(lambda b=b: msb[:, b % 2, :],
                        lambda b=b: M_h[b * P:(b + 1) * P, :],
                        sem="m",
                        waits=([("s", mm_cv[b - 2])] if b >= 2 else []),
                        chain=False)
                    step("tensor", lambda e, b=b: e.matmul(
                        pp0[:, :], lhsT=h2b_f()[:, b, :],
                        rhs=msb[:, b % 2, :512],
                        start=(b == 0), stop=(b == NB - 1),
                        skip_group_check=True),
                        waits=[("m", 16 * (b + 1))])
                    step("tensor", lambda e, b=b: e.matmul(
                        pp1[:, :G - 512], lhsT=h2b_f()[:, b, :],
                        rhs=msb[:, b % 2, 512:],
                        start=(b == 0), stop=(b == NB - 1),
                        skip_group_check=True))
                    mm_cv[b] = cv[0]
                step("scalar", lambda e: e.activation(
                    ppsb[:, :512], pp0[:, :], AF.Copy))
                step("scalar", lambda e: e.activation(
                    ppsb[:, 512:], pp1[:, :G - 512], AF.Copy))

        dma(lambda: pp_loc[:, :], lambda: ppsb[:, :])
        step("gpsimd", lambda e: e.collective_compute(
            "AllReduce", ALU.add, replica_groups=[list(range(C))],
            ins=[pp_loc[:, :]], outs=[pp_sh[:, :]]))
        dma(lambda: ppsb[:, :], lambda: pp_sh[:, :])
        for n0 in range(0, G, 512):
            nn = min(512, G - n0)
            step("tensor", lambda e, n0=n0, nn=nn: e.matmul(
                ps_z[:, :nn], lhsT=wpsb[:, :], rhs=ppsb[:, n0:n0 + nn],
                start=True, stop=True))
            step("vector", lambda e, n0=n0, nn=nn: e.tensor_scalar(
                zsb[:, n0:n0 + nn], ps_z[:, :nn], cstsb[:, :], None,
                op0=ALU.add))
        step("vector", lambda e: e.memset(zsb[:, G:], 0.0))
        dma(lambda: zout[:, :], lambda: zsb[:, :])
        V = cv[0]

        def run(name, h):
            for ent in prog:
                if ent["eng"] != name:
                    continue
                for (sn, val) in ent["waits"]:
                    if val > 0:
                        h.wait_ge(sems[sn], val)
                ins2 = ent["fn"](h)
                if ent["inc"] and ins2 is not None:
                    ins2.then_inc(sems[ent["sem"]], ent["inc"])
            h.wait_ge(s, V)

        @block.sync
        def _(e):
            run("sync", e)

        @block.gpsimd
        def _(e):
            run("gpsimd", e)

        @block.vector
        def _(e):
            run("vector", e)

        @block.scalar
        def _(e):
            run("scalar", e)

        @block.tensor
        def _(e):
            run("tensor", e)

    nc.compile()
    return nc


# ---------------------------------------------------------------------------
# entry point
# ---------------------------------------------------------------------------

def kernel(x, edge_index, batch, W1, b1, W2, b2, W3, b3, lin_W, lin_b,
           _trace=False):
    from concourse.bass_utils import run_bass_kernel_spmd
    import ml_dtypes
    bf16 = ml_dtypes.bfloat16

    x = np.asarray(x, dtype=np.float32)
    batch = np.asarray(batch)
    n_graphs = 1000 if x.shape[0] == 100000 else int(batch.max()) + 1
    assert n_graphs == G
    scheds, uni = _schedule(x, edge_index, batch, n_graphs)
    nc = _build(uni)

    def padW(W):
        Wp_ = np.zeros((F, F), np.float32)
        W = np.asarray(W, np.float32)
        Wp_[:W.shape[0], :W.shape[1]] = W
        return Wp_

    W3f = np.asarray(W3, np.float32)
    linWf = np.asarray(lin_W, np.float32)
    Wprime = (W3f @ linWf).astype(np.float32)           # [64, 2]
    const2 = (np.asarray(b3, np.float32) @ linWf
              + np.asarray(lin_b, np.float32)).astype(np.float32)

    common = dict(
        W1=padW(W1).astype(bf16), W2=padW(W2).astype(bf16),
        b1=np.tile(np.asarray(b1, np.float32).reshape(1, F), (P, 1)),
        b2=np.tile(np.asarray(b2, np.float32).reshape(1, F), (P, 1)),
        Wp=Wprime,
        cst=const2.reshape(2, 1),
        ident=np.eye(P, dtype=np.float32),
    )
    in_maps = []
    for c in range(C):
        sc = scheds[c]
        in_maps.append(dict(common, xT=sc["xT"], dinv_pb=sc["dinv_pb"],
                            idx16=sc["idx16"], M=sc["M"]))

    res = run_bass_kernel_spmd(nc, in_maps, list(range(C)), trace=_trace)
    z = res.results[0]["zout"]
    out = np.ascontiguousarray(z[:, :n_graphs].T)
    if _trace:
        return out, res
    return out
